# revision 11
# baseline (speedup 1.0000x reference)
"""Causal STFT kernel for Trainium2 (8 NeuronCores, data-parallel over batch).

Problem: x [16, 524288] f32 -> mag [16, 513, 2048] f32.
  Per batch: causal pad 1023 zeros on the left, frames of 1024 at hop 256
  (2048 frames), multiply by Hann-windowed DFT basis (1026 x 1024), take
  per-bin magnitude sqrt(re^2 + im^2).

Sharding: batch dim split 2 per core across 8 cores (SPMD, no collectives).

Device strategy (v2):
  - Host relayouts each padded signal into C_h[p, c] = xp[256c + 128h + p]
    and a partition-reversed copy D_g[p, c] = xp[256c - 128g - p], fp16,
    shipped as one packed tensor cd[b, p, g, c] (one DMA per chunk).
  - Window symmetry folds Fplus = C + D, Fminus = C - D halve the PE
    contraction to K = 512 (see _pack_weight_fold).  Folds are chunked per
    512-frame n-tile with one-tile lookahead so the PE never starves.
  - PE p-state is prewarmed with dummy matmuls on a memset scratch tile so
    real matmuls run at full clock from the start.
  - Magnitude pipeline is spread over three engines: ACT squares the cos
    PSUM and takes the final sqrt, DVE squares half the sin PSUMs (fp16
    out) and does the fp16 adds at 4x rate, Pool squares the other half
    and issues half the output DMAs.  The eps clip is dropped (only
    affects |X| < 1e-6, negligible vs rel-err tolerance).
"""

import os
import sys

import numpy as np

for _p in ("/opt/trn_rl_repo",):
    if _p not in sys.path and os.path.isdir(_p):
        sys.path.insert(0, _p)

N_FFT = 1024
HOP = 256
CACHE = N_FFT - 1  # 1023 zeros of causal left pad
BATCH = 16
SAMPLES = HOP * 2048
L = 2048  # frames per batch
F = 513  # output bins per batch
NCORES = 8
BPC = BATCH // NCORES  # batches per core = 2
NCHUNK = (CACHE + SAMPLES + 1) // HOP  # 2052 chunks of 256 after padding
NT = L // 512  # 4 frame tiles
QT = 4  # 4 (re, im) pair tiles of 128 bins

MODE = "v2"
N_PREWARM = 12  # dummy matmuls to ramp the PE p-state before real work

_PROGRAM_CACHE = {}


def _build_program_v2():
    import concourse.bacc as bacc
    import concourse.mybir as mybir
    import concourse.tile as tile

    f32 = mybir.dt.float32
    f16 = mybir.dt.float16

    nc = bacc.Bacc("TRN2", target_bir_lowering=False, debug=False)
    # weights packed flat: wp_a at col a*513 (513 cols each: 512 cos bins of
    # chunk a + the bin-512 column), wm_a at 4*513 + a*512
    w_in = nc.declare_dram_parameter("w", [128, 4 * 513 + 4 * 512], f16, isOutput=False)
    # signal layouts packed p-major: cd[b, p, g, c], g = (c0, c1, d0, d1)
    cd_in = nc.declare_dram_parameter("cd", [BPC, 128, 4, NCHUNK], f16, isOutput=False)
    out = nc.declare_dram_parameter("out", [BPC, F, L], f32, isOutput=True)

    WPOFF = [a * 513 for a in range(4)]
    WMOFF = [4 * 513 + a * 512 for a in range(4)]

    with tile.TileContext(nc) as tc:
        with (
            tc.tile_pool(name="wtp", bufs=1) as wtp,
            tc.tile_pool(name="cdp", bufs=2) as cdp,
            tc.tile_pool(name="fp", bufs=2) as fp,
            tc.tile_pool(name="scrp", bufs=1) as scrp,
            tc.tile_pool(name="pcp", bufs=2, space="PSUM") as pcp,
            tc.tile_pool(name="psp", bufs=3, space="PSUM") as psp,
            tc.tile_pool(name="p512p", bufs=1, space="PSUM") as p512p,
            tc.tile_pool(name="sqcp", bufs=3) as sqcp,
            tc.tile_pool(name="cpbp", bufs=3) as cpbp,
            tc.tile_pool(name="sqsp", bufs=3) as sqsp,
            tc.tile_pool(name="sp", bufs=3) as sp,
            tc.tile_pool(name="stp", bufs=3) as stp,
            tc.tile_pool(name="r512p", bufs=2) as r512p,
        ):
            # --- PE prewarm: dummy matmuls on a zeroed scratch tile ---
            scr = scrp.tile([128, 512], f16, name="scr")
            nc.gpsimd.memset(scr[:], 0.0)
            for i in range(N_PREWARM):
                pd = pcp.tile([128, 1024], f32, name=f"pd{i}", tag="pc")
                nc.tensor.matmul(
                    pd[:, 0:512], scr[:, 0:128], scr[:], start=True, stop=True
                )

            # --- input DMAs, spread across engine queues for an early start ---
            w_sb = wtp.tile([128, 4 * 513 + 4 * 512], f16, name="w")
            cd_sb = [
                cdp.tile([128, 4, NCHUNK], f16, name=f"cd{b}", tag="cd")
                for b in range(BPC)
            ]
            # chunk boundaries: fold tile n needs cols < 516 + 512*n
            CB = [0, 516, 1032, 2052]
            nc.gpsimd.dma_start(cd_sb[0][:, :, CB[0] : CB[1]], cd_in[0, :, :, CB[0] : CB[1]])
            nc.scalar.dma_start(w_sb[:], w_in[:])
            nc.scalar.dma_start(cd_sb[0][:, :, CB[1] : CB[2]], cd_in[0, :, :, CB[1] : CB[2]])
            nc.sync.dma_start(cd_sb[0][:, :, CB[2] : CB[3]], cd_in[0, :, :, CB[2] : CB[3]])
            nc.sync.dma_start(cd_sb[1][:], cd_in[1])

            def wp_q(a, q):
                return w_sb[:, WPOFF[a] + q * 128 : WPOFF[a] + (q + 1) * 128]

            def wp_512(a):
                return w_sb[:, WPOFF[a] + 512 : WPOFF[a] + 513]

            def wm_q(a, q):
                return w_sb[:, WMOFF[a] + q * 128 : WMOFF[a] + (q + 1) * 128]

            # --- fold tiles ---
            fpl = [[None] * 4 for _ in range(BPC)]
            fmi = [[None] * 4 for _ in range(BPC)]

            def c_sl(b, g, lo, hi):
                return cd_sb[b][:, g, lo:hi]

            def d_sl(b, g, lo, hi):
                return cd_sb[b][:, 2 + g, lo:hi]

            def fold_chunk(b, n):
                """Fold frames [512n, 512n+512) of batch b.  Even-offset
                reads (a=0,1: DVE 4x eligible) plus the center-sample copies
                run on DVE; odd-offset reads (a=2,3, which drop DVE to 2x
                anyway) go to the otherwise idle Pool engine."""
                if fpl[b][0] is None:
                    for a in range(4):
                        fpl[b][a] = fp.tile([128, L], f16, name=f"fp{b}{a}", tag=f"fp{a}")
                        fmi[b][a] = fp.tile([128, L], f16, name=f"fm{b}{a}", tag=f"fm{a}")
                lo, hi = n * 512, (n + 1) * 512
                for sign in range(2):
                    dst = fpl if sign == 0 else fmi
                    op = mybir.AluOpType.add if sign == 0 else mybir.AluOpType.subtract
                    for a in range(4):
                        g = a & 1
                        ao = a >> 1
                        eng = nc.vector if ao == 0 else nc.gpsimd
                        eng.tensor_tensor(
                            dst[b][a][:, lo:hi],
                            c_sl(b, g, lo + ao, hi + ao),
                            d_sl(b, g, lo + 4 - ao, hi + 4 - ao),
                            op=op,
                        )
                    # slot m=0 carries the self-paired center sample x[512]
                    nc.vector.tensor_copy(
                        dst[b][0][0:1, lo:hi], cd_sb[b][0:1, 0, lo + 2 : hi + 2]
                    )

            groups = [(b, n) for b in range(BPC) for n in range(NT)]
            fold_chunk(*groups[0])
            fold_chunk(*groups[1])

            for gi, (b, n) in enumerate(groups):
                nsl = slice(n * 512, (n + 1) * 512)
                last = gi == len(groups) - 1

                # --- PE: bin-512 strip first, then interleaved cos/sin ---
                p512 = p512p.tile([1, 512], f32, name=f"p512{b}{n}", tag="p512")
                for a in range(4):
                    nc.tensor.matmul(
                        p512[:], wp_512(a), fpl[b][a][:, nsl],
                        start=(a == 0), stop=(a == 3),
                    )
                # cos pairs go into 2-bank-wide PSUM tiles so ACT can drain
                # two q's per instruction; sin pairs stay 1-bank for DVE.
                pc_t, ps_t = [], []
                for h in range(2):
                    pc = pcp.tile([128, 1024], f32, name=f"pc{b}{n}{h}", tag="pc")
                    pc_t.append(pc)
                    for j in range(2):
                        q = 2 * h + j
                        for a in range(4):
                            nc.tensor.matmul(
                                pc[:, j * 512 : (j + 1) * 512],
                                wp_q(a, q), fpl[b][a][:, nsl],
                                start=(a == 0), stop=(a == 3),
                            )
                        ps = psp.tile([128, 512], f32, name=f"ps{b}{n}{q}", tag="ps")
                        for a in range(4):
                            nc.tensor.matmul(
                                ps[:], wm_q(a, q), fmi[b][a][:, nsl],
                                start=(a == 0), stop=(a == 3),
                            )
                        ps_t.append(ps)

                # fold lookahead: keep DVE one n-tile ahead of the PE
                if gi + 2 < len(groups):
                    fold_chunk(*groups[gi + 2])

                # --- bin 512: |re_512| on ACT, DMA from gpsimd queue ---
                r512 = r512p.tile([1, 512], f32, name=f"r512{b}{n}", tag="r512")
                nc.scalar.activation(r512[:], p512[:], mybir.ActivationFunctionType.Abs)
                nc.gpsimd.dma_start(out[b, F - 1 : F, nsl], r512[:])

                # --- magnitude pipeline ---
                # ACT drains the wide cos PSUMs (fused square -> fp16 SBUF)
                # and takes the final sqrt; DVE drains the sin PSUMs (fp16
                # copies, TensorTensor cannot read two PSUM operands), then
                # squares and adds the fp16 pairs at 4x SBUF rate.
                for h in range(2):  # q pairs (2h, 2h+1)
                    sqc = sqcp.tile([128, 1024], f16, name=f"sqc{b}{n}{h}", tag="sqc")
                    cpb = cpbp.tile([128, 1024], f16, name=f"cpb{b}{n}{h}", tag="cpb")
                    nc.scalar.square(sqc[:], pc_t[h][:])
                    for j in range(2):
                        q = 2 * h + j
                        nc.vector.tensor_copy(
                            cpb[:, j * 512 : (j + 1) * 512], ps_t[q][:]
                        )
                    sqs = sqsp.tile([128, 1024], f16, name=f"sqs{b}{n}{h}", tag="sqs")
                    nc.vector.tensor_tensor(
                        sqs[:], cpb[:], cpb[:], op=mybir.AluOpType.mult
                    )
                    s = sp.tile([128, 1024], f16, name=f"s{b}{n}{h}", tag="s")
                    # sin bin-0 row is all zero, so row 0 gives |re_0| = bin 0.
                    nc.vector.tensor_tensor(
                        s[:], sqc[:], sqs[:], op=mybir.AluOpType.add
                    )
                    st = stp.tile([128, 1024], f32, name=f"st{b}{n}{h}", tag="st")
                    nc.scalar.sqrt(st[:], s[:])
                    for j in range(2):
                        q = 2 * h + j
                        nc.sync.dma_start(
                            out[b, q * 128 : (q + 1) * 128, nsl],
                            st[:, j * 512 : (j + 1) * 512],
                        )
    nc.finalize()
    return nc


def _get_program():
    key = MODE
    if key not in _PROGRAM_CACHE:
        _PROGRAM_CACHE[key] = _build_program_v2()
    return _PROGRAM_CACHE[key]


def _make_weight_np():
    n = np.arange(N_FFT, dtype=np.float32)
    k = np.arange(N_FFT // 2 + 1, dtype=np.float32)[:, None]
    ang = (-2.0 * np.pi / N_FFT) * k * n[None, :]
    win = 0.5 * (1.0 - np.cos(2.0 * np.pi * n / N_FFT))
    return np.concatenate([np.cos(ang), np.sin(ang)], axis=0) * win  # [1026, 1024]


def _pack_weight_fold(weight):
    if weight is None:
        w2 = _make_weight_np()
    else:
        w2 = np.asarray(weight, dtype=np.float32).reshape(2 * (N_FFT // 2 + 1), N_FFT)
    # fold column j contracts x[j] + x[1024-j] (j = 1..511); slot j=0 carries
    # the center sample x[512], whose weight column is w2[:, 512].
    colmap = np.concatenate([[512], np.arange(1, 512)])
    wplus = w2[0:513][:, colmap]  # cos bins 0..512  [513, 512]
    wminus = w2[513:1025][:, colmap]  # sin bins 0..511 (row 0 zero)  [512, 512]
    wp = np.ascontiguousarray(wplus.T.reshape(4, 128, 513)).astype(np.float16)
    wm = np.ascontiguousarray(wminus.T.reshape(4, 128, 512)).astype(np.float16)
    # flat layout: wp_a at col a*513, wm_a at 4*513 + a*512
    w_flat = np.empty((128, 4 * 513 + 4 * 512), dtype=np.float16)
    for a in range(4):
        w_flat[:, a * 513 : (a + 1) * 513] = wp[a]
        w_flat[:, 4 * 513 + a * 512 : 4 * 513 + (a + 1) * 512] = wm[a]
    return w_flat


def _frame_layout(xb):
    """[SAMPLES] f32 -> C[2, 128, NCHUNK] with C[h, p, c] = xp[256c + 128h + p]."""
    xp = np.empty(NCHUNK * HOP, dtype=np.float32)
    xp[:CACHE] = 0.0
    xp[CACHE : CACHE + SAMPLES] = xb
    xp[CACHE + SAMPLES :] = 0.0
    return np.ascontiguousarray(xp.reshape(NCHUNK, 2, 128).transpose(1, 2, 0))


def _frame_layout_rev(xb):
    """Partition-reversed copy: D[g, p, c] = xp[256c - 128g - p] (0 if oob)."""
    xp = np.empty(NCHUNK * HOP, dtype=np.float32)
    xp[:CACHE] = 0.0
    xp[CACHE : CACHE + SAMPLES] = xb
    xp[CACHE + SAMPLES :] = 0.0
    c = 256 * np.arange(NCHUNK, dtype=np.int64)[None, None, :]
    g = 128 * np.arange(2, dtype=np.int64)[:, None, None]
    p = np.arange(128, dtype=np.int64)[None, :, None]
    idx = c - g - p
    d = xp[np.clip(idx, 0, None)]
    d[idx < 0] = 0.0
    return np.ascontiguousarray(d)


def _pack_cd(xb):
    """[SAMPLES] -> cd[128, 4, NCHUNK] fp16, g = (c0, c1, d0, d1)."""
    cmat = _frame_layout(xb)  # [2, 128, NCHUNK]
    dmat = _frame_layout_rev(xb)  # [2, 128, NCHUNK]
    cd = np.concatenate([cmat, dmat], axis=0)  # [4, 128, NCHUNK]
    return np.ascontiguousarray(cd.transpose(1, 0, 2)).astype(np.float16)


def _in_maps(x, weight):
    w_flat = _pack_weight_fold(weight)
    maps = []
    for i in range(NCORES):
        cd = np.stack([_pack_cd(x[BPC * i + b]) for b in range(BPC)])
        maps.append({"w": w_flat, "cd": cd})
    return maps


def kernel(x, weight=None, **_unused):
    from concourse.bass_utils import run_bass_kernel_spmd

    x = np.asarray(x, dtype=np.float32)
    assert x.shape == (BATCH, SAMPLES), x.shape

    nc = _get_program()
    res = run_bass_kernel_spmd(nc, _in_maps(x, weight), core_ids=list(range(NCORES)))

    out = np.empty((BATCH, F, L), dtype=np.float32)
    for i in range(NCORES):
        out[BPC * i : BPC * (i + 1)] = res.results[i]["out"]
    return out


# revision 31
# speedup vs baseline: 1.1105x; 1.1105x over previous
"""Causal STFT kernel for Trainium2 (8 NeuronCores, data-parallel over batch).

Problem: x [16, 524288] f32 -> mag [16, 513, 2048] f32.
  Per batch: causal pad 1023 zeros on the left, frames of 1024 at hop 256
  (2048 frames), multiply by Hann-windowed DFT basis (1026 x 1024), take
  per-bin magnitude sqrt(re^2 + im^2).

Sharding: batch dim split 2 per core across 8 cores (SPMD, no collectives).

Device strategy (v5):
  - Host relayouts each padded signal into C_h[p, c] = xp[256c + 128h + p]
    and a partition-reversed copy D_g[p, c] = xp[256c - 128g - p], fp16.
    A small pre-sliced starter tensor (first 520 columns, contiguous ->
    4KB DMA packets) lands in ~3us so the PE can start early; the full
    tensors follow as per-plane DMAs (4KB packets).  DMA packet cost is
    ~350ns + size/26GB/s per engine, so packet size dominates bandwidth.
  - Window symmetry folds Fplus = C + D, Fminus = C - D halve the PE
    contraction to K = 512 (see _pack_weight_fold).  Folds are chunked per
    512-frame n-tile with lookahead, all on DVE (running elementwise work
    on Pool concurrently with DVE slows both ~3x via SBUF contention).
    The self-paired center sample x[512] (zero-weight pair-0 slot) is
    DMA'd straight from DRAM into partition row 0 after each fold chunk.
  - PE p-state is prewarmed with dummy matmuls on a memset scratch tile.
  - Magnitude: ACT drains the cos PSUM pairs ([128,1024] two-bank reads,
    fused square to fp16) and sin q0/q1 (narrow squares); DVE drains sin
    q2/q3 (fp16 casts + fp16 square; TensorTensor cannot read two PSUM
    operands), does the fp16 adds, ACT takes the final sqrt into
    per-(b,q) full-row strip tiles [128, 2048] (8KB DRAM rows).
  - Output DMAs write half strips [128,1024] (4KB rows) after n-tiles 1
    and 3, spread across the sync/scalar/gpsimd rings so ring credits
    don't serialize the drain, overlapping compute.
  - The eps clip of the reference only affects |X| < 1e-6 and is dropped.
"""

import os
import sys

import numpy as np

for _p in ("/opt/trn_rl_repo",):
    if _p not in sys.path and os.path.isdir(_p):
        sys.path.insert(0, _p)

N_FFT = 1024
HOP = 256
CACHE = N_FFT - 1  # 1023 zeros of causal left pad
BATCH = 16
SAMPLES = HOP * 2048
L = 2048  # frames per batch
F = 513  # output bins per batch
NCORES = 8
BPC = BATCH // NCORES  # batches per core = 2
NCHUNK = (CACHE + SAMPLES + 1) // HOP  # 2052 chunks of 256 after padding
NT = L // 512  # 4 frame tiles
QT = 4  # 4 (re, im) pair tiles of 128 bins
NC0 = 520  # starter tensor columns (fold chunk 0 reads cols < 520)

MODE = "v5"
N_PREWARM = 12  # dummy matmuls to ramp the PE p-state before real work

_PROGRAM_CACHE = {}


def _build_program_v5():
    import concourse.bacc as bacc
    import concourse.mybir as mybir
    import concourse.tile as tile

    f32 = mybir.dt.float32
    f16 = mybir.dt.float16

    nc = bacc.Bacc("TRN2", target_bir_lowering=False, debug=False)
    # weights packed flat: wp_a at col a*513 (512 cos bins of chunk a + the
    # bin-512 column), wm_a at 4*513 + a*512
    w_in = nc.declare_dram_parameter("w", [128, 4 * 513 + 4 * 512], f16, isOutput=False)
    # signal layouts p-major: cd[b, p, g, c], g = (c0, c1, d0, d1); cd0 is
    # the pre-sliced starter (first NC0 columns, contiguous per partition)
    cd_in = nc.declare_dram_parameter("cd", [BPC, 128, 4, NCHUNK], f16, isOutput=False)
    cd0_in = nc.declare_dram_parameter("cd0", [BPC, 128, 4, NC0], f16, isOutput=False)
    # center samples ctr[b, 0, t] = xp[256t + 512] (frame t's center)
    ctr_in = nc.declare_dram_parameter("ctr", [BPC, 1, L], f16, isOutput=False)
    out = nc.declare_dram_parameter("out", [BPC, F, L], f32, isOutput=True)

    WPOFF = [a * 513 for a in range(4)]
    WMOFF = [4 * 513 + a * 512 for a in range(4)]

    with tile.TileContext(nc) as tc:
        with (
            tc.tile_pool(name="wtp", bufs=1) as wtp,
            tc.tile_pool(name="cdp", bufs=2) as cdp,
            tc.tile_pool(name="cd0p", bufs=1) as cd0p,
            tc.tile_pool(name="fp", bufs=2) as fp,
            tc.tile_pool(name="scrp", bufs=1) as scrp,
            tc.tile_pool(name="pcp", bufs=2, space="PSUM") as pcp,
            tc.tile_pool(name="psp", bufs=3, space="PSUM") as psp,
            tc.tile_pool(name="p512p", bufs=1, space="PSUM") as p512p,
            tc.tile_pool(name="sqcp", bufs=2) as sqcp,
            tc.tile_pool(name="cpbp", bufs=2) as cpbp,
            tc.tile_pool(name="sqsp", bufs=2) as sqsp,
            tc.tile_pool(name="sp", bufs=2) as sp,
            tc.tile_pool(name="stfp", bufs=2) as stfp,
            tc.tile_pool(name="r512p", bufs=1) as r512p,
        ):
            # --- PE prewarm: dummy matmuls on a zeroed scratch tile ---
            scr = scrp.tile([128, 512], f16, name="scr")
            nc.gpsimd.memset(scr[:], 0.0)
            for i in range(N_PREWARM):
                pd = pcp.tile([128, 1024], f32, name=f"pd{i}", tag="pc")
                nc.tensor.matmul(
                    pd[:, 0:512], scr[:, 0:128], scr[:], start=True, stop=True
                )

            # --- input DMAs: starters first for an early PE start, then
            # full-width per-plane loads (4KB packets each) ---
            w_sb = wtp.tile([128, 4 * 513 + 4 * 512], f16, name="w")
            cd_sb = [
                cdp.tile([128, 4, NCHUNK], f16, name=f"cd{b}", tag="cd")
                for b in range(BPC)
            ]
            # only batch 0 needs the starter; batch 1's full planes land
            # long before its first fold is due
            cd0_sb = [cd0p.tile([128, 4, NC0], f16, name="cd00", tag="cd0"), None]
            nc.gpsimd.dma_start(cd0_sb[0][:], cd0_in[0])
            nc.scalar.dma_start(w_sb[:], w_in[:])
            for b in range(BPC):
                for g in range(4):
                    eng = nc.sync if b == 0 else (nc.scalar if g < 2 else nc.gpsimd)
                    eng.dma_start(cd_sb[b][:, g, :], cd_in[b, :, g, :])

            def wp_q(a, q):
                return w_sb[:, WPOFF[a] + q * 128 : WPOFF[a] + (q + 1) * 128]

            def wp_512(a):
                return w_sb[:, WPOFF[a] + 512 : WPOFF[a] + 513]

            def wm_q(a, q):
                return w_sb[:, WMOFF[a] + q * 128 : WMOFF[a] + (q + 1) * 128]

            # --- fold tiles ---
            fpl = [[None] * 4 for _ in range(BPC)]
            fmi = [[None] * 4 for _ in range(BPC)]

            def fold_chunk(b, n):
                """Fold frames [512n, 512n+512) of batch b on DVE; chunk 0
                reads the starter tile so it does not wait for the full
                load.  Row 0 of the a=0 tiles (zero-weight pair-0 slot) is
                then overwritten with the center samples via a tiny DMA."""
                if fpl[b][0] is None:
                    for a in range(4):
                        fpl[b][a] = fp.tile([128, L], f16, name=f"fp{b}{a}", tag=f"fp{a}")
                        fmi[b][a] = fp.tile([128, L], f16, name=f"fm{b}{a}", tag=f"fm{a}")
                lo, hi = n * 512, (n + 1) * 512
                src = cd0_sb[b] if (n == 0 and cd0_sb[b] is not None) else cd_sb[b]
                for sign in range(2):
                    dst = fpl if sign == 0 else fmi
                    op = mybir.AluOpType.add if sign == 0 else mybir.AluOpType.subtract
                    for a in range(4):
                        g = a & 1
                        ao = a >> 1
                        nc.vector.tensor_tensor(
                            dst[b][a][:, lo:hi],
                            src[:, g, lo + ao : hi + ao],
                            src[:, 2 + g, lo + 4 - ao : hi + 4 - ao],
                            op=op,
                        )
                    nc.gpsimd.dma_start(
                        dst[b][0][0:1, lo:hi], ctr_in[b, 0:1, lo:hi]
                    )

            groups = [(b, n) for b in range(BPC) for n in range(NT)]
            fold_chunk(*groups[0])
            fold_chunk(*groups[1])

            # per-(b,q) full-row output strips; r512 strip per b
            stf = [[None] * QT for _ in range(BPC)]
            r512 = [
                r512p.tile([1, L], f32, name=f"r512{b}", tag=f"r512{b}")
                for b in range(BPC)
            ]

            for gi, (b, n) in enumerate(groups):
                nsl = slice(n * 512, (n + 1) * 512)
                last = gi == len(groups) - 1
                if n == 0:
                    for q in range(QT):
                        stf[b][q] = stfp.tile(
                            [128, L], f32, name=f"stf{b}{q}", tag=f"stf{q}"
                        )

                # --- PE: bin-512 strip first, then interleaved cos/sin ---
                p512 = p512p.tile([1, 512], f32, name=f"p512{b}{n}", tag="p512")
                for a in range(4):
                    nc.tensor.matmul(
                        p512[:], wp_512(a), fpl[b][a][:, nsl],
                        start=(a == 0), stop=(a == 3),
                    )
                # cos pairs go into 2-bank-wide PSUM tiles so ACT can drain
                # two q's per instruction; sin pairs stay 1-bank.
                pc_t, ps_t = [], []
                for h in range(2):
                    pc = pcp.tile([128, 1024], f32, name=f"pc{b}{n}{h}", tag="pc")
                    pc_t.append(pc)
                    for j in range(2):
                        q = 2 * h + j
                        for a in range(4):
                            nc.tensor.matmul(
                                pc[:, j * 512 : (j + 1) * 512],
                                wp_q(a, q), fpl[b][a][:, nsl],
                                start=(a == 0), stop=(a == 3),
                            )
                        ps = psp.tile([128, 512], f32, name=f"ps{b}{n}{q}", tag="ps")
                        for a in range(4):
                            nc.tensor.matmul(
                                ps[:], wm_q(a, q), fmi[b][a][:, nsl],
                                start=(a == 0), stop=(a == 3),
                            )
                        ps_t.append(ps)

                # fold lookahead: keep DVE one n-tile ahead of the PE
                if gi + 2 < len(groups):
                    fold_chunk(*groups[gi + 2])

                # --- bin 512: |re_512| on ACT into the strip ---
                nc.scalar.activation(
                    r512[b][0:1, nsl], p512[:], mybir.ActivationFunctionType.Abs
                )

                # --- magnitude: ACT drains cos pairs + two sins, DVE drains
                # the other two sins and adds; ACT takes the final sqrt.
                # In the last group ACT takes q2/q3 (they finish last on the
                # PE) so the tail chain stays on the faster path.
                act_h = 1 if last else 0  # h pair whose sins go to ACT
                sqc_t, sqs_t = [], []
                for h in range(2):
                    sqc = sqcp.tile([128, 1024], f16, name=f"sqc{b}{n}{h}", tag="sqc")
                    nc.scalar.square(sqc[:], pc_t[h][:])
                    sqc_t.append(sqc)
                    sqs = sqsp.tile([128, 1024], f16, name=f"sqs{b}{n}{h}", tag="sqs")
                    if h == act_h:
                        for j in range(2):
                            nc.scalar.square(
                                sqs[:, j * 512 : (j + 1) * 512], ps_t[2 * h + j][:]
                            )
                    else:
                        cpb = cpbp.tile(
                            [128, 1024], f16, name=f"cpb{b}{n}{h}", tag="cpb"
                        )
                        for j in range(2):
                            nc.vector.tensor_copy(
                                cpb[:, j * 512 : (j + 1) * 512], ps_t[2 * h + j][:]
                            )
                        nc.vector.tensor_tensor(
                            sqs[:], cpb[:], cpb[:], op=mybir.AluOpType.mult
                        )
                    sqs_t.append(sqs)
                for h in range(2):
                    s = sp.tile([128, 1024], f16, name=f"s{b}{n}{h}", tag="s")
                    # sin bin-0 row is all zero, so row 0 gives |re_0| = bin 0
                    nc.vector.tensor_tensor(
                        s[:], sqc_t[h][:], sqs_t[h][:], op=mybir.AluOpType.add
                    )
                    for j in range(2):
                        q = 2 * h + j
                        nc.scalar.sqrt(
                            stf[b][q][:, nsl], s[:, j * 512 : (j + 1) * 512]
                        )

                # --- output: half strips (4KB rows) after n=1 and n=3,
                # spread across the sync/scalar/gpsimd rings ---
                if n in (1, 3):
                    hsl = slice((n - 1) * 512, (n + 1) * 512)
                    for q in range(QT):
                        eng = nc.sync if q < 2 else nc.scalar
                        eng.dma_start(
                            out[b, q * 128 : (q + 1) * 128, hsl], stf[b][q][:, hsl]
                        )
                    nc.gpsimd.dma_start(out[b, F - 1 : F, hsl], r512[b][0:1, hsl])
    nc.finalize()
    return nc


def _get_program():
    key = MODE
    if key not in _PROGRAM_CACHE:
        _PROGRAM_CACHE[key] = _build_program_v5()
    return _PROGRAM_CACHE[key]


def _make_weight_np():
    n = np.arange(N_FFT, dtype=np.float32)
    k = np.arange(N_FFT // 2 + 1, dtype=np.float32)[:, None]
    ang = (-2.0 * np.pi / N_FFT) * k * n[None, :]
    win = 0.5 * (1.0 - np.cos(2.0 * np.pi * n / N_FFT))
    return np.concatenate([np.cos(ang), np.sin(ang)], axis=0) * win  # [1026, 1024]


def _pack_weight_fold(weight):
    if weight is None:
        w2 = _make_weight_np()
    else:
        w2 = np.asarray(weight, dtype=np.float32).reshape(2 * (N_FFT // 2 + 1), N_FFT)
    # fold column j contracts x[j] + x[1024-j] (j = 1..511); slot j=0 carries
    # the center sample x[512], whose weight column is w2[:, 512].
    colmap = np.concatenate([[512], np.arange(1, 512)])
    wplus = w2[0:513][:, colmap]  # cos bins 0..512  [513, 512]
    wminus = w2[513:1025][:, colmap]  # sin bins 0..511 (row 0 zero)  [512, 512]
    wp = np.ascontiguousarray(wplus.T.reshape(4, 128, 513)).astype(np.float16)
    wm = np.ascontiguousarray(wminus.T.reshape(4, 128, 512)).astype(np.float16)
    # flat layout: wp_a at col a*513, wm_a at 4*513 + a*512
    w_flat = np.empty((128, 4 * 513 + 4 * 512), dtype=np.float16)
    for a in range(4):
        w_flat[:, a * 513 : (a + 1) * 513] = wp[a]
        w_flat[:, 4 * 513 + a * 512 : 4 * 513 + (a + 1) * 512] = wm[a]
    return w_flat


def _padded(xb):
    xp = np.empty(NCHUNK * HOP, dtype=np.float32)
    xp[:CACHE] = 0.0
    xp[CACHE : CACHE + SAMPLES] = xb
    xp[CACHE + SAMPLES :] = 0.0
    return xp


def _frame_layout(xp):
    """C[2, 128, NCHUNK] with C[h, p, c] = xp[256c + 128h + p]."""
    return np.ascontiguousarray(xp.reshape(NCHUNK, 2, 128).transpose(1, 2, 0))


def _frame_layout_rev(xp):
    """Partition-reversed copy: D[g, p, c] = xp[256c - 128g - p] (0 if oob)."""
    c = 256 * np.arange(NCHUNK, dtype=np.int64)[None, None, :]
    g = 128 * np.arange(2, dtype=np.int64)[:, None, None]
    p = np.arange(128, dtype=np.int64)[None, :, None]
    idx = c - g - p
    d = xp[np.clip(idx, 0, None)]
    d[idx < 0] = 0.0
    return np.ascontiguousarray(d)


def _pack_cd(xb):
    """[SAMPLES] -> (cd[128, 4, NCHUNK], cd0[128, 4, NC0], ctr[1, L]) fp16."""
    xp = _padded(xb)
    cmat = _frame_layout(xp)  # [2, 128, NCHUNK]
    dmat = _frame_layout_rev(xp)  # [2, 128, NCHUNK]
    cd = np.concatenate([cmat, dmat], axis=0)  # [4, 128, NCHUNK]
    cd = np.ascontiguousarray(cd.transpose(1, 0, 2)).astype(np.float16)
    cd0 = np.ascontiguousarray(cd[:, :, :NC0])
    ctr = np.ascontiguousarray(
        xp[512 : 512 + L * HOP : HOP].astype(np.float16)[None, :]
    )  # ctr[t] = xp[256t + 512]
    return cd, cd0, ctr


def _in_maps(x, weight):
    w_flat = _pack_weight_fold(weight)
    maps = []
    for i in range(NCORES):
        packed = [_pack_cd(x[BPC * i + b]) for b in range(BPC)]
        maps.append(
            {
                "w": w_flat,
                "cd": np.stack([p[0] for p in packed]),
                "cd0": np.stack([p[1] for p in packed]),
                "ctr": np.stack([p[2] for p in packed]),
            }
        )
    return maps


def kernel(x, weight=None, **_unused):
    from concourse.bass_utils import run_bass_kernel_spmd

    x = np.asarray(x, dtype=np.float32)
    assert x.shape == (BATCH, SAMPLES), x.shape

    nc = _get_program()
    res = run_bass_kernel_spmd(nc, _in_maps(x, weight), core_ids=list(range(NCORES)))

    out = np.empty((BATCH, F, L), dtype=np.float32)
    for i in range(NCORES):
        out[BPC * i : BPC * (i + 1)] = res.results[i]["out"]
    return out
